# revision 8
# baseline (speedup 1.0000x reference)
"""Class-conditional BatchNorm2d (eval path, alpha=0.5) on 8 Trainium2 cores.

Strategy (data-parallel over batch, per the sharding hint):
  - Each of the 8 cores gets 16 of the 128 samples; the small stat tables
    (global/class running mean/var, weight, bias) are replicated.
  - On-device, per core:
      1. one-hot(labels) built with iota + is_equal, transposed [class, b]
      2. PE matmul gathers class stats:  meanT[c,b] = class_mean[labels[b], c]
      3. interpolate with global stats, sqrt+reciprocal -> inv_std
      4. scaleT[c,b] = inv_std*weight[c]; shiftT[c,b] = bias[c]-mean*scale
      5. stream each sample [128ch x 3136px] through one fused
         tensor_scalar (x*scale + shift) with per-partition scalars.
  - Memory-bound: 49 MiB HBM traffic per core (load + store), compute hides
    underneath the DMA.
"""

import numpy as np
from contextlib import ExitStack

import concourse.bacc as bacc
import concourse.tile as tile
from concourse import mybir
from concourse.bass_utils import run_bass_kernel_spmd

B, C, H, W = 128, 128, 56, 56
HW = H * W
NCORES = 8
BS = B // NCORES  # 16 samples per core
NCLS = 100
EPS = 1e-5
ALPHA = 0.5

F32 = mybir.dt.float32
I32 = mybir.dt.int32

_CACHED_NC = None


def _build_nc():
    nc = bacc.Bacc(
        "TRN2",
        debug=False,
        enable_asserts=False,
        target_bir_lowering=False,
        num_devices=NCORES,
    )

    x_d = nc.dram_tensor("x", [BS // 2, 2, C, HW], F32, kind="ExternalInput")
    lbl_d = nc.dram_tensor("labels", [1, BS], I32, kind="ExternalInput")
    w_d = nc.dram_tensor("weight", [C, 1], F32, kind="ExternalInput")
    b_d = nc.dram_tensor("bias", [C, 1], F32, kind="ExternalInput")
    gm_d = nc.dram_tensor("gmean", [C, 1], F32, kind="ExternalInput")
    gv_d = nc.dram_tensor("gvar", [C, 1], F32, kind="ExternalInput")
    cm_d = nc.dram_tensor("cmean", [NCLS, C], F32, kind="ExternalInput")
    cv_d = nc.dram_tensor("cvar", [NCLS, C], F32, kind="ExternalInput")
    out_d = nc.dram_tensor("out", [BS // 2, 2, C, HW], F32, kind="ExternalOutput")

    with tile.TileContext(nc) as tc, ExitStack() as ctx:
        const = ctx.enter_context(tc.tile_pool(name="const", bufs=1))
        psum = ctx.enter_context(tc.tile_pool(name="psum", bufs=1, space="PSUM"))
        data = ctx.enter_context(tc.tile_pool(name="data", bufs=5))

        # ---- small tables (head of the scalar HWDGE ring: arms early,
        # done in a few us, and doesn't delay the big loads that stream
        # on the sync ring from t=0) ----
        cm_sb = const.tile([NCLS, C], F32)
        nc.scalar.dma_start(cm_sb[:], cm_d.ap())
        cv_sb = const.tile([NCLS, C], F32)
        nc.scalar.dma_start(cv_sb[:], cv_d.ap())
        w_col = const.tile([C, 1], F32)
        nc.scalar.dma_start(w_col[:], w_d.ap())
        b_col = const.tile([C, 1], F32)
        nc.scalar.dma_start(b_col[:], b_d.ap())
        gm_col = const.tile([C, 1], F32)
        nc.scalar.dma_start(gm_col[:], gm_d.ap())
        gv_col = const.tile([C, 1], F32)
        nc.scalar.dma_start(gv_col[:], gv_d.ap())
        lbl_i = const.tile([1, BS], I32)
        nc.scalar.dma_start(lbl_i[:], lbl_d.ap())

        # labels -> f32
        lbl_f = const.tile([1, BS], F32)
        nc.vector.tensor_copy(lbl_f[:], lbl_i[:])

        # broadcast labels across all 128 partitions via a K=1 matmul
        ones_row = const.tile([1, C], F32)
        nc.vector.memset(ones_row[:], 1.0)
        lbl_bc = psum.tile([C, BS], F32)
        nc.tensor.matmul(lbl_bc[:], ones_row[:], lbl_f[:], start=True, stop=True)

        # iota over partitions -> one-hot^T[k, b] = (labels[b] == k)
        iota_i = const.tile([C, 1], I32)
        nc.gpsimd.iota(iota_i[:], pattern=[[0, 1]], base=0, channel_multiplier=1)
        iota_f = const.tile([C, 1], F32)
        nc.vector.tensor_copy(iota_f[:], iota_i[:])
        onehotT = const.tile([C, BS], F32)
        nc.vector.tensor_scalar(
            onehotT[:], lbl_bc[:], iota_f[:], None, mybir.AluOpType.is_equal
        )

        # gather class stats: statT[c, b] = class_stat[labels[b], c]
        meanT_cls = psum.tile([C, BS], F32)
        nc.tensor.matmul(
            meanT_cls[:], cm_sb[:], onehotT[:NCLS, :], start=True, stop=True
        )
        varT_cls = psum.tile([C, BS], F32)
        nc.tensor.matmul(
            varT_cls[:], cv_sb[:], onehotT[:NCLS, :], start=True, stop=True
        )

        # interpolate with global stats: alpha*class + (1-alpha)*global
        gm_half = const.tile([C, 1], F32)
        nc.scalar.mul(gm_half[:], gm_col[:], 1.0 - ALPHA)
        gv_half = const.tile([C, 1], F32)
        nc.scalar.mul(gv_half[:], gv_col[:], 1.0 - ALPHA)

        meanT = const.tile([C, BS], F32)
        nc.vector.tensor_scalar(
            meanT[:], meanT_cls[:], ALPHA, gm_half[:],
            mybir.AluOpType.mult, mybir.AluOpType.add,
        )
        varT = const.tile([C, BS], F32)
        nc.vector.tensor_scalar(
            varT[:], varT_cls[:], ALPHA, gv_half[:],
            mybir.AluOpType.mult, mybir.AluOpType.add,
        )

        # inv_std = 1/sqrt(var + eps)
        eps_col = const.tile([C, 1], F32)
        nc.vector.memset(eps_col[:], EPS)
        stdT = const.tile([C, BS], F32)
        nc.scalar.activation(
            stdT[:], varT[:], mybir.ActivationFunctionType.Sqrt, bias=eps_col[:]
        )
        invT = const.tile([C, BS], F32)
        nc.vector.reciprocal(invT[:], stdT[:])

        # scale = inv_std * weight ; shift = bias - mean * scale
        scaleT = const.tile([C, BS], F32)
        nc.vector.tensor_scalar(
            scaleT[:], invT[:], w_col[:], None, mybir.AluOpType.mult
        )
        msc = const.tile([C, BS], F32)
        nc.vector.tensor_tensor(msc[:], meanT[:], scaleT[:], mybir.AluOpType.mult)
        shiftT = const.tile([C, BS], F32)
        nc.vector.tensor_scalar(
            shiftT[:], msc[:], -1.0, b_col[:],
            mybir.AluOpType.mult, mybir.AluOpType.add,
        )

        # ---- stream the samples: out = x*scale + shift ----
        # 2 samples per DMA (3.2 MiB) to amortize per-dma_start ring
        # stalls; loads and stores alternate between the two HWDGE rings
        # (sync + scalar) so both directions flow continuously and one
        # store's HBM-completion stall hides under the other ring's
        # transfers.
        for t in range(BS // 2):
            ld_eng = nc.sync if t % 2 == 0 else nc.scalar
            st_eng = nc.scalar if t % 2 == 0 else nc.sync
            xt = data.tile([C, 2, HW], F32)
            ld_eng.dma_start(xt[:], x_d.ap()[t].transpose([1, 0, 2]))
            for s in range(2):
                i = 2 * t + s
                nc.vector.tensor_scalar(
                    xt[:, s, :], xt[:, s, :],
                    scaleT[:, i : i + 1], shiftT[:, i : i + 1],
                    mybir.AluOpType.mult, mybir.AluOpType.add,
                )
            st_eng.dma_start(out_d.ap()[t].transpose([1, 0, 2]), xt[:])

    nc.compile()
    return nc


def _get_nc():
    global _CACHED_NC
    if _CACHED_NC is None:
        _CACHED_NC = _build_nc()
    return _CACHED_NC


def _make_in_maps(inputs):
    x = np.ascontiguousarray(np.asarray(inputs["x"], dtype=np.float32)).reshape(
        B, C, HW
    )
    labels = np.asarray(inputs["labels"]).astype(np.int32)
    w = np.asarray(inputs["weight"], dtype=np.float32).reshape(C, 1)
    b = np.asarray(inputs["bias"], dtype=np.float32).reshape(C, 1)
    gm = np.asarray(inputs["global_running_mean"], dtype=np.float32).reshape(C, 1)
    gv = np.asarray(inputs["global_running_var"], dtype=np.float32).reshape(C, 1)
    cm = np.ascontiguousarray(
        np.asarray(inputs["class_running_mean"], dtype=np.float32)
    )
    cv = np.ascontiguousarray(
        np.asarray(inputs["class_running_var"], dtype=np.float32)
    )

    in_maps = []
    for i in range(NCORES):
        sl = slice(i * BS, (i + 1) * BS)
        in_maps.append(
            {
                "x": np.ascontiguousarray(x[sl]).reshape(BS // 2, 2, C, HW),
                "labels": np.ascontiguousarray(labels[sl]).reshape(1, BS),
                "weight": w,
                "bias": b,
                "gmean": gm,
                "gvar": gv,
                "cmean": cm,
                "cvar": cv,
            }
        )
    return in_maps


def _run(inputs, trace=False, **kwargs):
    nc = _get_nc()
    in_maps = _make_in_maps(inputs)
    return run_bass_kernel_spmd(
        nc, in_maps, list(range(NCORES)), trace=trace, **kwargs
    )


def kernel(**inputs) -> np.ndarray:
    res = _run(inputs, trace=False)
    out = np.empty((B, C, H, W), dtype=np.float32)
    for i in range(NCORES):
        out[i * BS : (i + 1) * BS] = res.results[i]["out"].reshape(BS, C, H, W)
    return out


# revision 10
# speedup vs baseline: 1.0310x; 1.0310x over previous
"""Class-conditional BatchNorm2d (eval path, alpha=0.5) on 8 Trainium2 cores.

Strategy (data-parallel over batch, per the sharding hint):
  - Each of the 8 cores gets 16 of the 128 samples; the small stat tables
    (global/class running mean/var, weight, bias) are replicated.
  - On-device, per core:
      1. one-hot(labels) built with iota + is_equal, transposed [class, b]
      2. PE matmul gathers class stats:  meanT[c,b] = class_mean[labels[b], c]
      3. interpolate with global stats, sqrt+reciprocal -> inv_std
      4. scaleT[c,b] = inv_std*weight[c]; shiftT[c,b] = bias[c]-mean*scale
      5. stream each sample [128ch x 3136px] through one fused
         tensor_scalar (x*scale + shift) with per-partition scalars.
  - Memory-bound: 49 MiB HBM traffic per core (load + store), compute hides
    underneath the DMA.
"""

import numpy as np
from contextlib import ExitStack

import concourse.bacc as bacc
import concourse.tile as tile
from concourse import mybir
from concourse.bass_utils import run_bass_kernel_spmd

B, C, H, W = 128, 128, 56, 56
HW = H * W
NCORES = 8
BS = B // NCORES  # 16 samples per core
NCLS = 100
EPS = 1e-5
ALPHA = 0.5

F32 = mybir.dt.float32
I32 = mybir.dt.int32

_CACHED_NC = None


def _build_nc():
    nc = bacc.Bacc(
        "TRN2",
        debug=False,
        enable_asserts=False,
        target_bir_lowering=False,
        num_devices=NCORES,
    )

    x_d = nc.dram_tensor("x", [BS // 2, 2, C, HW], F32, kind="ExternalInput")
    lbl_d = nc.dram_tensor("labels", [1, BS], I32, kind="ExternalInput")
    w_d = nc.dram_tensor("weight", [C, 1], F32, kind="ExternalInput")
    b_d = nc.dram_tensor("bias", [C, 1], F32, kind="ExternalInput")
    gm_d = nc.dram_tensor("gmean", [C, 1], F32, kind="ExternalInput")
    gv_d = nc.dram_tensor("gvar", [C, 1], F32, kind="ExternalInput")
    cm_d = nc.dram_tensor("cmean", [NCLS, C], F32, kind="ExternalInput")
    cv_d = nc.dram_tensor("cvar", [NCLS, C], F32, kind="ExternalInput")
    out_d = nc.dram_tensor("out", [BS // 2, 2, C, HW], F32, kind="ExternalOutput")

    with tile.TileContext(nc) as tc, ExitStack() as ctx:
        const = ctx.enter_context(tc.tile_pool(name="const", bufs=1))
        psum = ctx.enter_context(tc.tile_pool(name="psum", bufs=1, space="PSUM"))
        data_a = ctx.enter_context(tc.tile_pool(name="data_a", bufs=3))
        data_b = ctx.enter_context(tc.tile_pool(name="data_b", bufs=3))

        # ---- small tables (head of the scalar HWDGE ring: arms early,
        # done in a few us, and doesn't delay the big loads that stream
        # on the sync ring from t=0) ----
        cm_sb = const.tile([NCLS, C], F32)
        nc.scalar.dma_start(cm_sb[:], cm_d.ap())
        cv_sb = const.tile([NCLS, C], F32)
        nc.scalar.dma_start(cv_sb[:], cv_d.ap())
        w_col = const.tile([C, 1], F32)
        nc.scalar.dma_start(w_col[:], w_d.ap())
        b_col = const.tile([C, 1], F32)
        nc.scalar.dma_start(b_col[:], b_d.ap())
        gm_col = const.tile([C, 1], F32)
        nc.scalar.dma_start(gm_col[:], gm_d.ap())
        gv_col = const.tile([C, 1], F32)
        nc.scalar.dma_start(gv_col[:], gv_d.ap())
        lbl_i = const.tile([1, BS], I32)
        nc.scalar.dma_start(lbl_i[:], lbl_d.ap())

        # labels -> f32
        lbl_f = const.tile([1, BS], F32)
        nc.vector.tensor_copy(lbl_f[:], lbl_i[:])

        # broadcast labels across all 128 partitions via a K=1 matmul
        ones_row = const.tile([1, C], F32)
        nc.vector.memset(ones_row[:], 1.0)
        lbl_bc = psum.tile([C, BS], F32)
        nc.tensor.matmul(lbl_bc[:], ones_row[:], lbl_f[:], start=True, stop=True)

        # iota over partitions -> one-hot^T[k, b] = (labels[b] == k)
        iota_i = const.tile([C, 1], I32)
        nc.gpsimd.iota(iota_i[:], pattern=[[0, 1]], base=0, channel_multiplier=1)
        iota_f = const.tile([C, 1], F32)
        nc.vector.tensor_copy(iota_f[:], iota_i[:])
        onehotT = const.tile([C, BS], F32)
        nc.vector.tensor_scalar(
            onehotT[:], lbl_bc[:], iota_f[:], None, mybir.AluOpType.is_equal
        )

        # gather class stats: statT[c, b] = class_stat[labels[b], c]
        meanT_cls = psum.tile([C, BS], F32)
        nc.tensor.matmul(
            meanT_cls[:], cm_sb[:], onehotT[:NCLS, :], start=True, stop=True
        )
        varT_cls = psum.tile([C, BS], F32)
        nc.tensor.matmul(
            varT_cls[:], cv_sb[:], onehotT[:NCLS, :], start=True, stop=True
        )

        # interpolate with global stats: alpha*class + (1-alpha)*global
        gm_half = const.tile([C, 1], F32)
        nc.scalar.mul(gm_half[:], gm_col[:], 1.0 - ALPHA)
        gv_half = const.tile([C, 1], F32)
        nc.scalar.mul(gv_half[:], gv_col[:], 1.0 - ALPHA)

        meanT = const.tile([C, BS], F32)
        nc.vector.tensor_scalar(
            meanT[:], meanT_cls[:], ALPHA, gm_half[:],
            mybir.AluOpType.mult, mybir.AluOpType.add,
        )
        varT = const.tile([C, BS], F32)
        nc.vector.tensor_scalar(
            varT[:], varT_cls[:], ALPHA, gv_half[:],
            mybir.AluOpType.mult, mybir.AluOpType.add,
        )

        # inv_std = 1/sqrt(var + eps)
        eps_col = const.tile([C, 1], F32)
        nc.vector.memset(eps_col[:], EPS)
        stdT = const.tile([C, BS], F32)
        nc.scalar.activation(
            stdT[:], varT[:], mybir.ActivationFunctionType.Sqrt, bias=eps_col[:]
        )
        invT = const.tile([C, BS], F32)
        nc.vector.reciprocal(invT[:], stdT[:])

        # scale = inv_std * weight ; shift = bias - mean * scale
        scaleT = const.tile([C, BS], F32)
        nc.vector.tensor_scalar(
            scaleT[:], invT[:], w_col[:], None, mybir.AluOpType.mult
        )
        msc = const.tile([C, BS], F32)
        nc.vector.tensor_tensor(msc[:], meanT[:], scaleT[:], mybir.AluOpType.mult)
        shiftT = const.tile([C, BS], F32)
        nc.vector.tensor_scalar(
            shiftT[:], msc[:], -1.0, b_col[:],
            mybir.AluOpType.mult, mybir.AluOpType.add,
        )

        # ---- stream the samples: out = x*scale + shift ----
        # Two independent interleaved streams, one per HWDGE ring: each
        # tile's load AND store ride the same ring (self-contained dep
        # chain, no cross-ring convoys), tiles alternate rings, and each
        # ring has its own slot pool. 2 samples per DMA (3.2 MiB)
        # amortizes per-dma_start ring overhead.
        for t in range(BS // 2):
            if t % 2 == 0:
                eng, pool = nc.sync, data_a
            else:
                eng, pool = nc.scalar, data_b
            xt = pool.tile([C, 2, HW], F32, name=f"xt_r{t % 2}")
            eng.dma_start(xt[:], x_d.ap()[t].transpose([1, 0, 2]))
            for s in range(2):
                i = 2 * t + s
                nc.vector.tensor_scalar(
                    xt[:, s, :], xt[:, s, :],
                    scaleT[:, i : i + 1], shiftT[:, i : i + 1],
                    mybir.AluOpType.mult, mybir.AluOpType.add,
                )
            eng.dma_start(out_d.ap()[t].transpose([1, 0, 2]), xt[:])

    nc.compile()
    return nc


def _get_nc():
    global _CACHED_NC
    if _CACHED_NC is None:
        _CACHED_NC = _build_nc()
    return _CACHED_NC


def _make_in_maps(inputs):
    x = np.ascontiguousarray(np.asarray(inputs["x"], dtype=np.float32)).reshape(
        B, C, HW
    )
    labels = np.asarray(inputs["labels"]).astype(np.int32)
    w = np.asarray(inputs["weight"], dtype=np.float32).reshape(C, 1)
    b = np.asarray(inputs["bias"], dtype=np.float32).reshape(C, 1)
    gm = np.asarray(inputs["global_running_mean"], dtype=np.float32).reshape(C, 1)
    gv = np.asarray(inputs["global_running_var"], dtype=np.float32).reshape(C, 1)
    cm = np.ascontiguousarray(
        np.asarray(inputs["class_running_mean"], dtype=np.float32)
    )
    cv = np.ascontiguousarray(
        np.asarray(inputs["class_running_var"], dtype=np.float32)
    )

    in_maps = []
    for i in range(NCORES):
        sl = slice(i * BS, (i + 1) * BS)
        in_maps.append(
            {
                "x": np.ascontiguousarray(x[sl]).reshape(BS // 2, 2, C, HW),
                "labels": np.ascontiguousarray(labels[sl]).reshape(1, BS),
                "weight": w,
                "bias": b,
                "gmean": gm,
                "gvar": gv,
                "cmean": cm,
                "cvar": cv,
            }
        )
    return in_maps


def _run(inputs, trace=False, **kwargs):
    nc = _get_nc()
    in_maps = _make_in_maps(inputs)
    return run_bass_kernel_spmd(
        nc, in_maps, list(range(NCORES)), trace=trace, **kwargs
    )


def kernel(**inputs) -> np.ndarray:
    res = _run(inputs, trace=False)
    out = np.empty((B, C, H, W), dtype=np.float32)
    for i in range(NCORES):
        out[i * BS : (i + 1) * BS] = res.results[i]["out"].reshape(BS, C, H, W)
    return out
